# revision 8
# baseline (speedup 1.0000x reference)
"""Entmax-1.5 over rows of a (2048, 32000) fp32 tensor on 8 Trainium2 NeuronCores.

Per row, with raw-units threshold c (y = relu((x - c)/2)^2, sum y = 1):
  1. SWDGE cast-DMA loads x as fp16 tiles per 128-row block (block A leads
     with a 2000-wide tile so the first packet moves sooner). Each tile folds
     independently (pairwise-halving strided max, groups of 16) to a G chunk;
     DVE max8 per subrange gives K=80 candidates.
  2. Warm Newton on candidates (fp32): free-window pre-iters on the first two
     tiles' candidates, then 3 full iters on all 80 after the last tile.
  3. relu pass in place on DVE (tensor_scalar 4x fp16).
  4. f0 = sum (r/2)^2: block A entirely on ScalarE Square-accum (DVE preps
     block B); block B: ScalarE tile-0 halves + DVE stt squares tiles 1-3.
  5. Newton on ScalarE: dc = max(0, (f0-1)*2*rs), nh = -dc/2.
  6. out pass: block A entirely ScalarE Square(0.5 r + nh) -> fp16 bounce
     (keeps DVE free for block B's chain); block B: DVE tiles 3,2,1
     (shift+self-mult in place) + ScalarE tile-0 halves.

Host: shard rows 8 ways, gather, cast fp16 -> fp32.
"""

import os
import numpy as np

import concourse.bass as bass
import concourse.bacc as bacc
import concourse.mybir as mybir
from concourse.tile import TileContext
from concourse.bass_utils import run_bass_kernel_spmd

f32 = mybir.dt.float32
f16 = mybir.dt.float16
Alu = mybir.AluOpType
Act = mybir.ActivationFunctionType
AxX = mybir.AxisListType.X

ROWS_TOTAL = 2048
V = 32000
N_CORES = 8
ROWS_PER_CORE = ROWS_TOTAL // N_CORES  # 256
P = 128

TILES_A = [2000, 16000, 8000, 6000]
TILES_B = [16000, 8000, 6000, 2000]
# fold target width / max8 subranges per tile width (groups of 16)
GCFG = {16000: (1000, 4), 8000: (500, 2), 6000: (375, 3), 2000: (125, 1)}
K = 80
WARM_PRE = int(os.environ.get("WARM_PRE", "3"))
WARM_POST = int(os.environ.get("WARM_POST", "3"))

# f0/out unit lists per block: (tile_idx, lo, w). SC = ScalarE, V = DVE.
UNITS_A = [(1, 0, 8000), (1, 8000, 8000), (2, 0, 8000), (3, 0, 6000),
           (0, 0, 2000)]
UNITS_B_SC = [(0, 0, 8000), (0, 8000, 8000)]
UNITS_B_V = [(1, 0, 8000), (2, 0, 6000), (3, 0, 2000)]


class _Blk:
    pass


def build_kernel(nc: bass.Bass):
    x = nc.dram_tensor("x", [ROWS_PER_CORE, V], f32, kind="ExternalInput").ap()
    y = nc.dram_tensor("y", [ROWS_PER_CORE, V], f16, kind="ExternalOutput").ap()

    with TileContext(nc) as tc:
        with (
            tc.tile_pool(name="data", bufs=2) as dpool,
            tc.tile_pool(name="fold", bufs=1) as gpool,
            tc.tile_pool(name="ybuf", bufs=2) as ypool,
            tc.tile_pool(name="trash", bufs=1) as tpool,
            tc.tile_pool(name="small", bufs=2) as spool,
        ):
            def sm(tag, cols=1, dt=f32):
                return spool.tile([P, cols], dt, tag=tag, name=tag)

            z0 = spool.tile([P, 1], f32, tag="z0", name="z0", bufs=1)
            nc.vector.memset(z0, 0.0)
            zb = z0.to_broadcast([P, K])

            def new_block(b, tiles):
                s = _Blk()
                s.rows = slice(b * P, (b + 1) * P)
                s.tiles = tiles
                s.coff = [0]  # candidate column offsets per tile
                for w in tiles:
                    s.coff.append(s.coff[-1] + 8 * GCFG[w][1])
                assert s.coff[-1] == K
                s.k_pre = s.coff[2]  # candidates from the first two tiles
                s.xt = []
                return s

            def load(s, name):
                with nc.named_scope(f"load{name}"):
                    off = 0
                    for w in s.tiles:
                        xt = dpool.tile([P, w], f16, tag=f"xt{w}", name="xt")
                        s.xt.append(xt)
                        nc.gpsimd.dma_start(out=xt, in_=x[s.rows, off:off + w])
                        off += w

            def glo_of(s, t, lo):
                return sum(s.tiles[:t]) + lo

            def fold_tile(s, t, name):
                """Fold tile t by pairwise halving into G, then max8 the
                subranges into VK."""
                with nc.named_scope(f"fold{name}{t}"):
                    G = s.G
                    w = s.tiles[t]
                    gw, nrg = GCFG[w]
                    h = w // 2
                    nc.vector.tensor_tensor(out=G[:, 0:h], in0=s.xt[t][:, 0:h],
                                            in1=s.xt[t][:, h:w], op=Alu.max)
                    while h > gw:
                        nh_ = h // 2
                        nc.vector.tensor_tensor(out=G[:, 0:nh_],
                                                in0=G[:, 0:nh_],
                                                in1=G[:, nh_:h], op=Alu.max)
                        h = nh_
                    W = gw // nrg
                    for i in range(nrg):
                        o = s.coff[t] + 8 * i
                        nc.vector.max(out=s.VK[:, o:o + 8],
                                      in_=G[:, W * i:W * (i + 1)])

            def warm_iters(s, width, iters):
                VKf, rV, rV2 = s.VKf, s.rV, s.rV2
                S, Q, rs, u, C = s.S, s.Q, s.rs, s.u, s.C
                for _ in range(iters):
                    nc.vector.scalar_tensor_tensor(
                        out=rV[:, :width], in0=VKf[:, :width], scalar=C,
                        in1=zb[:, :width], op0=Alu.subtract, op1=Alu.max,
                        accum_out=S)
                    nc.vector.scalar_tensor_tensor(
                        out=rV2[:, :width], in0=rV[:, :width], scalar=1.0,
                        in1=rV[:, :width], op0=Alu.mult, op1=Alu.mult,
                        accum_out=Q)
                    nc.vector.reciprocal(rs, S)
                    nc.vector.scalar_tensor_tensor(
                        out=u, in0=Q, scalar=4.0, in1=rs,
                        op0=Alu.subtract, op1=Alu.mult)
                    nc.vector.scalar_tensor_tensor(
                        out=C, in0=u, scalar=0.5, in1=C,
                        op0=Alu.mult, op1=Alu.add)

            def warm_pre(s, name):
                with nc.named_scope(f"warmpre{name}"):
                    kp = s.k_pre
                    nc.vector.tensor_copy(s.VKf[:, :kp], s.VK[:, :kp])
                    vsum = sm("vsum")
                    nc.vector.tensor_reduce(out=vsum, in_=s.VKf[:, :kp],
                                            axis=AxX, op=Alu.add)
                    nc.vector.tensor_scalar_mul(s.C, vsum, 1.0 / kp)
                    warm_iters(s, kp, WARM_PRE)

            def warm_post(s, name):
                with nc.named_scope(f"warm{name}"):
                    nc.vector.tensor_copy(s.VKf, s.VK)
                    warm_iters(s, K, WARM_POST)
                    nc.vector.tensor_scalar_mul(s.nrsig, s.rs, -1.0)

            def relu_unit(s, t, lo, w):
                sl = slice(lo, lo + w)
                nc.vector.tensor_scalar(
                    out=s.xt[t][:, sl], in0=s.xt[t][:, sl],
                    scalar1=s.C, scalar2=0.0,
                    op0=Alu.subtract, op1=Alu.max)

            def f0_sc_unit(s, t, lo, w, col):
                yb = ypool.tile([P, 8000], f16, tag="yb", name="yb")
                nc.scalar.activation(
                    out=yb[:, :w], in_=s.xt[t][:, lo:lo + w],
                    func=Act.Square, scale=0.5,
                    accum_out=s.f0c[:, col:col + 1])

            def f0_v_unit(s, t, lo, w, col):
                tr = tpool.tile([P, 8000], f16, tag="tr", name="tr")
                nc.vector.scalar_tensor_tensor(
                    out=tr[:, :w], in0=s.xt[t][:, lo:lo + w], scalar=0.25,
                    in1=s.xt[t][:, lo:lo + w], op0=Alu.mult, op1=Alu.mult,
                    accum_out=s.f0c[:, col:col + 1])

            def newton_reduce(s):
                s.f0 = sm("f0")
                nc.vector.tensor_reduce(out=s.f0, in_=s.f0c, axis=AxX,
                                        op=Alu.add)

            def newton_sc(s, name):
                with nc.named_scope(f"newt{name}"):
                    dc0, dc, nh = sm("dc0"), sm("dc"), sm("nh")
                    nc.scalar.activation(out=dc0, in_=s.f0, func=Act.Identity,
                                         scale=s.rs, bias=s.nrsig)
                    nc.scalar.activation(out=dc, in_=dc0, func=Act.Relu,
                                         scale=2.0)
                    nc.scalar.activation(out=nh, in_=dc, func=Act.Identity,
                                         scale=-0.5)
                    s.dc, s.nh = dc, nh

            def out_sc_unit(s, t, lo, w):
                yb = ypool.tile([P, 8000], f16, tag="yb", name="yb")
                nc.scalar.activation(out=yb[:, :w], in_=s.xt[t][:, lo:lo + w],
                                     func=Act.Square, scale=0.5, bias=s.nh)
                nc.sync.dma_start(out=y[s.rows, glo_of(s, t, lo):
                                        glo_of(s, t, lo) + w],
                                  in_=yb[:, :w])

            def out_v_unit(s, t, lo, w):
                sl = slice(lo, lo + w)
                nc.vector.tensor_scalar(
                    out=s.xt[t][:, sl], in0=s.xt[t][:, sl],
                    scalar1=s.dc, scalar2=0.5,
                    op0=Alu.subtract, op1=Alu.mult)
                nc.vector.tensor_tensor(
                    out=s.xt[t][:, sl], in0=s.xt[t][:, sl],
                    in1=s.xt[t][:, sl], op=Alu.mult)
                nc.sync.dma_start(out=y[s.rows, glo_of(s, t, lo):
                                        glo_of(s, t, lo) + w],
                                  in_=s.xt[t][:, sl])

            def alloc_blk(s):
                s.G = gpool.tile([P, 8000], f16, tag="G", name="G")
                s.VK = spool.tile([P, K], f16, tag="VK", name="VK")
                s.VKf = sm("VKf", K)
                s.rV, s.rV2 = sm("rV", K), sm("rV2", K)
                s.S, s.Q, s.rs, s.u, s.C = (sm("S"), sm("Q"), sm("rs"),
                                            sm("u"), sm("C"))
                s.nrsig = sm("nrsig")
                s.f0c = sm("f0c", 5)

            A = new_block(0, TILES_A)
            B = new_block(1, TILES_B)
            load(A, "A")
            load(B, "B")

            # ---- block A threshold chain ----
            alloc_blk(A)
            fold_tile(A, 0, "A")
            fold_tile(A, 1, "A")
            warm_pre(A, "A")
            fold_tile(A, 2, "A")
            fold_tile(A, 3, "A")
            warm_post(A, "A")
            with nc.named_scope("reluA"):
                for (t, lo, w) in UNITS_A:
                    relu_unit(A, t, lo, w)
            with nc.named_scope("f0scA"):   # f0 A entirely on ScalarE
                for col, (t, lo, w) in enumerate(UNITS_A):
                    f0_sc_unit(A, t, lo, w, col)

            # ---- block B prep on DVE (overlaps A's ScalarE work) ----
            alloc_blk(B)
            fold_tile(B, 0, "B")
            fold_tile(B, 1, "B")
            newton_reduce(A)                 # tiny DVE op; feeds newtA
            newton_sc(A, "A")
            warm_pre(B, "B")
            fold_tile(B, 2, "B")
            fold_tile(B, 3, "B")
            warm_post(B, "B")
            # outA entirely on ScalarE (keeps DVE free for the B chain)
            with nc.named_scope("outscA"):
                for (t, lo, w) in UNITS_A:
                    out_sc_unit(A, t, lo, w)
            with nc.named_scope("reluB"):
                for (t, lo, w) in UNITS_B_SC + UNITS_B_V:
                    relu_unit(B, t, lo, w)
            with nc.named_scope("f0B"):
                for col, (t, lo, w) in enumerate(UNITS_B_SC):
                    f0_sc_unit(B, t, lo, w, col)          # ScalarE
                for col, (t, lo, w) in enumerate(UNITS_B_V):
                    f0_v_unit(B, t, lo, w, 2 + col)       # DVE stt
            newton_reduce(B)
            newton_sc(B, "B")
            with nc.named_scope("outB"):
                out_v_unit(B, 3, 0, 2000)
                out_sc_unit(B, 0, 0, 8000)
                out_v_unit(B, 2, 0, 6000)
                out_sc_unit(B, 0, 8000, 8000)
                out_v_unit(B, 1, 0, 8000)
    return nc


_COMPILED = {}


def _get_nc():
    if "nc" not in _COMPILED:
        nc = bacc.Bacc("TRN2", target_bir_lowering=False, debug=False,
                       num_devices=N_CORES)
        build_kernel(nc)
        nc.compile()
        _COMPILED["nc"] = nc
    return _COMPILED["nc"]


def kernel(X: np.ndarray) -> np.ndarray:
    assert X.shape == (ROWS_TOTAL, V) and X.dtype == np.float32, (X.shape, X.dtype)
    nc = _get_nc()
    in_maps = [
        {"x": np.ascontiguousarray(X[i * ROWS_PER_CORE:(i + 1) * ROWS_PER_CORE])}
        for i in range(N_CORES)
    ]
    res = run_bass_kernel_spmd(nc, in_maps, core_ids=list(range(N_CORES)))
    return np.concatenate(
        [r["y"].astype(np.float32) for r in res.results], axis=0)


# revision 9
# speedup vs baseline: 1.0333x; 1.0333x over previous
"""Entmax-1.5 over rows of a (2048, 32000) fp32 tensor on 8 Trainium2 NeuronCores.

Per row, with raw-units threshold c (y = relu((x - c)/2)^2, sum y = 1):
  1. SWDGE cast-DMA loads x as fp16 tiles (16000/8000/6000/2000 per 128-row
     block). Each tile folds independently (pairwise-halving strided max,
     groups of 16) to a G chunk; DVE max8 per subrange gives K=80 candidates.
  2. Warm Newton on candidates (fp32): free-window pre-iters on the first two
     tiles' 48 candidates, then 3 full iters on all 80 after the last tile.
  3. relu pass in place on DVE (tensor_scalar 4x fp16).
  4. f0 = sum (r/2)^2: block A entirely on ScalarE Square-accum (DVE preps
     block B); block B: ScalarE tiles 0-1 + DVE stt squares tiles 2-3.
  5. Newton (high-priority so the scheduler runs it the moment f0 lands):
     dc = max(0, (f0-1)*2*rs), nh = -dc/2.
  6. out pass: block A on ScalarE Square(0.5 r + nh) for tiles 0-1 plus DVE
     tiles 2-3 slotted between reluB and f0B; block B: DVE tiles 3,2,1 +
     ScalarE tile-0 halves, smallest units first to feed the DMA stream.

Host: shard rows 8 ways, gather, cast fp16 -> fp32.
"""

import os
import numpy as np

import concourse.bass as bass
import concourse.bacc as bacc
import concourse.mybir as mybir
from concourse.tile import TileContext
from concourse.bass_utils import run_bass_kernel_spmd

f32 = mybir.dt.float32
f16 = mybir.dt.float16
Alu = mybir.AluOpType
Act = mybir.ActivationFunctionType
AxX = mybir.AxisListType.X

ROWS_TOTAL = 2048
V = 32000
N_CORES = 8
ROWS_PER_CORE = ROWS_TOTAL // N_CORES  # 256
P = 128

TILES = [16000, 8000, 6000, 2000]
GCFG = {16000: (1000, 4), 8000: (500, 2), 6000: (375, 3), 2000: (125, 1)}
K = 80
K_PRE = 48
WARM_PRE = int(os.environ.get("WARM_PRE", "3"))
WARM_POST = int(os.environ.get("WARM_POST", "3"))

# unit lists: (tile_idx, lo, w)
U_T0A, U_T0B = (0, 0, 8000), (0, 8000, 8000)
U_T1, U_T2, U_T3 = (1, 0, 8000), (2, 0, 6000), (3, 0, 2000)
ALL_UNITS = [U_T0A, U_T0B, U_T1, U_T2, U_T3]


class _Blk:
    pass


def build_kernel(nc: bass.Bass):
    x = nc.dram_tensor("x", [ROWS_PER_CORE, V], f32, kind="ExternalInput").ap()
    y = nc.dram_tensor("y", [ROWS_PER_CORE, V], f16, kind="ExternalOutput").ap()

    with TileContext(nc) as tc:
        with (
            tc.tile_pool(name="data", bufs=2) as dpool,
            tc.tile_pool(name="fold", bufs=1) as gpool,
            tc.tile_pool(name="ybuf", bufs=2) as ypool,
            tc.tile_pool(name="trash", bufs=1) as tpool,
            tc.tile_pool(name="small", bufs=2) as spool,
        ):
            def sm(tag, cols=1, dt=f32):
                return spool.tile([P, cols], dt, tag=tag, name=tag)

            z0 = spool.tile([P, 1], f32, tag="z0", name="z0", bufs=1)
            nc.vector.memset(z0, 0.0)
            zb = z0.to_broadcast([P, K])

            def new_block(b):
                s = _Blk()
                s.rows = slice(b * P, (b + 1) * P)
                s.coff = [0]
                for w in TILES:
                    s.coff.append(s.coff[-1] + 8 * GCFG[w][1])
                s.xt = []
                return s

            def load(s, name):
                with nc.named_scope(f"load{name}"):
                    off = 0
                    for w in TILES:
                        xt = dpool.tile([P, w], f16, tag=f"xt{w}", name="xt")
                        s.xt.append(xt)
                        nc.gpsimd.dma_start(out=xt, in_=x[s.rows, off:off + w])
                        off += w

            def fold_tile(s, t, name):
                with nc.named_scope(f"fold{name}{t}"):
                    G = s.G
                    w = TILES[t]
                    gw, nrg = GCFG[w]
                    h = w // 2
                    nc.vector.tensor_tensor(out=G[:, 0:h], in0=s.xt[t][:, 0:h],
                                            in1=s.xt[t][:, h:w], op=Alu.max)
                    while h > gw:
                        nh_ = h // 2
                        nc.vector.tensor_tensor(out=G[:, 0:nh_],
                                                in0=G[:, 0:nh_],
                                                in1=G[:, nh_:h], op=Alu.max)
                        h = nh_
                    W = gw // nrg
                    for i in range(nrg):
                        o = s.coff[t] + 8 * i
                        nc.vector.max(out=s.VK[:, o:o + 8],
                                      in_=G[:, W * i:W * (i + 1)])

            def warm_iters(s, width, iters):
                VKf, rV, rV2 = s.VKf, s.rV, s.rV2
                S, Q, rs, u, C = s.S, s.Q, s.rs, s.u, s.C
                for _ in range(iters):
                    nc.vector.scalar_tensor_tensor(
                        out=rV[:, :width], in0=VKf[:, :width], scalar=C,
                        in1=zb[:, :width], op0=Alu.subtract, op1=Alu.max,
                        accum_out=S)
                    nc.vector.scalar_tensor_tensor(
                        out=rV2[:, :width], in0=rV[:, :width], scalar=1.0,
                        in1=rV[:, :width], op0=Alu.mult, op1=Alu.mult,
                        accum_out=Q)
                    nc.vector.reciprocal(rs, S)
                    nc.vector.scalar_tensor_tensor(
                        out=u, in0=Q, scalar=4.0, in1=rs,
                        op0=Alu.subtract, op1=Alu.mult)
                    nc.vector.scalar_tensor_tensor(
                        out=C, in0=u, scalar=0.5, in1=C,
                        op0=Alu.mult, op1=Alu.add)

            def warm_pre(s, name):
                with nc.named_scope(f"warmpre{name}"):
                    nc.vector.tensor_copy(s.VKf[:, :K_PRE], s.VK[:, :K_PRE])
                    vsum = sm("vsum")
                    nc.vector.tensor_reduce(out=vsum, in_=s.VKf[:, :K_PRE],
                                            axis=AxX, op=Alu.add)
                    nc.vector.tensor_scalar_mul(s.C, vsum, 1.0 / K_PRE)
                    warm_iters(s, K_PRE, WARM_PRE)

            def warm_post(s, name):
                with nc.named_scope(f"warm{name}"):
                    nc.vector.tensor_copy(s.VKf, s.VK)
                    warm_iters(s, K, WARM_POST)
                    nc.vector.tensor_scalar_mul(s.nrsig, s.rs, -1.0)

            def relu_unit(s, t, lo, w):
                sl = slice(lo, lo + w)
                nc.vector.tensor_scalar(
                    out=s.xt[t][:, sl], in0=s.xt[t][:, sl],
                    scalar1=s.C, scalar2=0.0,
                    op0=Alu.subtract, op1=Alu.max)

            def f0_sc_unit(s, t, lo, w, col, trash=None):
                if trash is None:
                    trash = ypool.tile([P, 8000], f16, tag="yb", name="yb")
                nc.scalar.activation(
                    out=trash[:, :w], in_=s.xt[t][:, lo:lo + w],
                    func=Act.Square, scale=0.5,
                    accum_out=s.f0c[:, col:col + 1])

            def f0_v_unit(s, t, lo, w, col):
                tr = tpool.tile([P, 8000], f16, tag="tr", name="tr")
                nc.vector.scalar_tensor_tensor(
                    out=tr[:, :w], in0=s.xt[t][:, lo:lo + w], scalar=0.25,
                    in1=s.xt[t][:, lo:lo + w], op0=Alu.mult, op1=Alu.mult,
                    accum_out=s.f0c[:, col:col + 1])

            def newton(s, name, ncols):
                with nc.named_scope(f"newt{name}"), tc.high_priority():
                    f0 = sm("f0")
                    nc.vector.tensor_reduce(out=f0, in_=s.f0c[:, :ncols],
                                            axis=AxX, op=Alu.add)
                    dc0, dc, nh = sm("dc0"), sm("dc"), sm("nh")
                    nc.scalar.activation(out=dc0, in_=f0, func=Act.Identity,
                                         scale=s.rs, bias=s.nrsig)
                    nc.scalar.activation(out=dc, in_=dc0, func=Act.Relu,
                                         scale=2.0)
                    nc.scalar.activation(out=nh, in_=dc, func=Act.Identity,
                                         scale=-0.5)
                    s.dc, s.nh = dc, nh

            def glo_of(s, t, lo):
                return sum(TILES[:t]) + lo

            def out_sc_unit(s, t, lo, w):
                yb = ypool.tile([P, 8000], f16, tag="yb", name="yb")
                nc.scalar.activation(out=yb[:, :w], in_=s.xt[t][:, lo:lo + w],
                                     func=Act.Square, scale=0.5, bias=s.nh)
                g = glo_of(s, t, lo)
                nc.sync.dma_start(out=y[s.rows, g:g + w], in_=yb[:, :w])

            def out_v_unit(s, t, lo, w):
                sl = slice(lo, lo + w)
                nc.vector.tensor_scalar(
                    out=s.xt[t][:, sl], in0=s.xt[t][:, sl],
                    scalar1=s.dc, scalar2=0.5,
                    op0=Alu.subtract, op1=Alu.mult)
                nc.vector.tensor_tensor(
                    out=s.xt[t][:, sl], in0=s.xt[t][:, sl],
                    in1=s.xt[t][:, sl], op=Alu.mult)
                g = glo_of(s, t, lo)
                nc.sync.dma_start(out=y[s.rows, g:g + w], in_=s.xt[t][:, sl])

            def alloc_blk(s):
                s.G = gpool.tile([P, 8000], f16, tag="G", name="G")
                s.VK = spool.tile([P, K], f16, tag="VK", name="VK")
                s.VKf = sm("VKf", K)
                s.rV, s.rV2 = sm("rV", K), sm("rV2", K)
                s.S, s.Q, s.rs, s.u, s.C = (sm("S"), sm("Q"), sm("rs"),
                                            sm("u"), sm("C"))
                s.nrsig = sm("nrsig")
                s.f0c = sm("f0c", 5)

            A, B = new_block(0), new_block(1)
            load(A, "A")
            load(B, "B")

            # ---- block A threshold chain ----
            alloc_blk(A)
            fold_tile(A, 0, "A")
            fold_tile(A, 1, "A")
            warm_pre(A, "A")
            fold_tile(A, 2, "A")
            fold_tile(A, 3, "A")
            warm_post(A, "A")
            with nc.named_scope("reluA"):
                for (t, lo, w) in ALL_UNITS:
                    relu_unit(A, t, lo, w)
            with nc.named_scope("f0scA"):   # f0 A entirely on ScalarE
                for col, (t, lo, w) in enumerate(ALL_UNITS):
                    f0_sc_unit(A, t, lo, w, col)

            # ---- block B prep on DVE (overlaps A's ScalarE chain) ----
            alloc_blk(B)
            fold_tile(B, 0, "B")
            fold_tile(B, 1, "B")
            newton(A, "A", 5)               # high-priority: runs when f0A lands
            warm_pre(B, "B")
            fold_tile(B, 2, "B")
            fold_tile(B, 3, "B")
            warm_post(B, "B")
            # outA ScalarE part feeds the output stream head
            with nc.named_scope("outscA"):
                out_sc_unit(A, 0, 0, 8000)
                out_sc_unit(A, 0, 8000, 8000)
                out_sc_unit(A, 1, 0, 8000)
            with nc.named_scope("reluB"):
                for (t, lo, w) in ALL_UNITS:
                    relu_unit(B, t, lo, w)
            # outA DVE part (deadlines are late in the stream)
            with nc.named_scope("outvA"):
                out_v_unit(A, 2, 0, 6000)
                out_v_unit(A, 3, 0, 2000)
            with nc.named_scope("f0B"):     # SC tiles 0-1, DVE stt tiles 2-3
                f0_sc_unit(B, 0, 0, 8000, 0, trash=B.G)
                f0_sc_unit(B, 0, 8000, 8000, 1, trash=B.G)
                f0_sc_unit(B, 1, 0, 8000, 2, trash=B.G)
                f0_v_unit(B, 2, 0, 6000, 3)
                f0_v_unit(B, 3, 0, 2000, 4)
            newton(B, "B", 5)
            with nc.named_scope("outB"):
                out_v_unit(B, 3, 0, 2000)
                out_v_unit(B, 2, 0, 6000)
                out_sc_unit(B, 0, 0, 8000)
                out_v_unit(B, 1, 0, 8000)
                out_sc_unit(B, 0, 8000, 8000)
    return nc


_COMPILED = {}


def _get_nc():
    if "nc" not in _COMPILED:
        nc = bacc.Bacc("TRN2", target_bir_lowering=False, debug=False,
                       num_devices=N_CORES)
        build_kernel(nc)
        nc.compile()
        _COMPILED["nc"] = nc
    return _COMPILED["nc"]


def kernel(X: np.ndarray) -> np.ndarray:
    assert X.shape == (ROWS_TOTAL, V) and X.dtype == np.float32, (X.shape, X.dtype)
    nc = _get_nc()
    in_maps = [
        {"x": np.ascontiguousarray(X[i * ROWS_PER_CORE:(i + 1) * ROWS_PER_CORE])}
        for i in range(N_CORES)
    ]
    res = run_bass_kernel_spmd(nc, in_maps, core_ids=list(range(N_CORES)))
    return np.concatenate(
        [r["y"].astype(np.float32) for r in res.results], axis=0)


# revision 11
# speedup vs baseline: 1.0949x; 1.0597x over previous
"""Entmax-1.5 over rows of a (2048, 32000) fp32 tensor on 8 Trainium2 NeuronCores.

Per row, with raw-units threshold c (y = relu((x - c)/2)^2, sum y = 1):
  1. SWDGE cast-DMA loads x as fp16 tiles (widths 16000/8000/6000/2000 per
     128-row block). Each tile folds independently on DVE (pairwise-halving
     strided max, groups of 16) to a G chunk, then DVE max8 per subrange
     gives K=80 candidates (32/16/24/8 per tile). Per-tile folding keeps the
     post-last-tile work tiny (one 1000-wide TT + one max8).
  2. Warm Newton on candidates (fp32): free-window pre-iters on the first
     two tiles' 48 candidates while tiles 2-3 stream, then 3 full iters on
     all 80 once the last tile lands -> c_w, with 1/S from the last iter.
  3. relu pass in place (DVE tensor_scalar 4x fp16, fused with f0 below).
  4. f0 = sum (r/2)^2 via ScalarE Square-accum chunks (ybuf/PSUM trash) plus
     DVE stt-square-accum chunks (dead fold buffer as trash).
  5. Newton on ScalarE: dc = max(0, (f0-1)*2/sig), nh = -dc/2.
  6. out pass split between ScalarE Square(0.5 r + nh) -> fp16 bounce and
     DVE shift+self-mult in place; DMA units to fp16 DRAM output.

Emission order is chosen for the in-order per-engine programs so block B's
fold/warm and block A's iter/out interleave without blocking each other.

Host: shard rows 8 ways, gather, cast fp16 -> fp32.
"""

import os
import numpy as np

import concourse.bass as bass
import concourse.bacc as bacc
import concourse.mybir as mybir
from concourse.tile import TileContext
from concourse.bass_utils import run_bass_kernel_spmd

f32 = mybir.dt.float32
f16 = mybir.dt.float16
Alu = mybir.AluOpType
Act = mybir.ActivationFunctionType
AxX = mybir.AxisListType.X

ROWS_TOTAL = 2048
V = 32000
N_CORES = 8
ROWS_PER_CORE = ROWS_TOTAL // N_CORES  # 256
P = 128
N_BLOCKS = 2
TILE_WS = [16000, 8000, 6000, 2000]    # per-block tile widths (sum 32000)
# per-tile fold target width and number of max8 subranges (groups of 16)
GCFG = {16000: (1000, 4), 8000: (500, 2), 6000: (375, 3), 2000: (125, 1)}
K = 80
K_PRE = 48                              # candidates from tiles 0-1
WARM_PRE = int(os.environ.get("WARM_PRE", "3"))
WARM_POST = int(os.environ.get("WARM_POST", "3"))
# out units per block: (tile, lo, width)
OUT_UNITS = [(0, 0, 8000), (0, 8000, 8000), (1, 0, 8000),
             (2, 0, 6000), (3, 0, 2000)]
# f0 computed per unit: units listed here go to DVE (stt square-accum into
# the dead fold buffer), the rest to ScalarE Square-accum (8000-wide units
# bounce via ybuf trash, smaller via PSUM trash).
F0_V_UNITS = tuple(int(c) for c in os.environ.get("F0V", "24"))
OUT_S_A = int(os.environ.get("OUT_S_A", "3"))        # A units on ScalarE
OUT_S_B = int(os.environ.get("OUT_S_B", "2"))        # B units on ScalarE


class _Blk:
    pass


def build_kernel(nc: bass.Bass):
    x = nc.dram_tensor("x", [ROWS_PER_CORE, V], f32, kind="ExternalInput").ap()
    y = nc.dram_tensor("y", [ROWS_PER_CORE, V], f16, kind="ExternalOutput").ap()

    with TileContext(nc) as tc:
        with (
            tc.tile_pool(name="data", bufs=2) as dpool,
            tc.tile_pool(name="fold", bufs=2) as gpool,
            tc.tile_pool(name="ybuf", bufs=2) as ypool,
            tc.tile_pool(name="small", bufs=2) as spool,
            tc.tile_pool(name="psum", bufs=1, space="PSUM") as ppool,
        ):
            def sm(tag, cols=1, dt=f32):
                return spool.tile([P, cols], dt, tag=tag, name=tag)

            z0 = spool.tile([P, 1], f32, tag="z0", name="z0", bufs=1)
            nc.vector.memset(z0, 0.0)
            zb = z0.to_broadcast([P, K])

            def new_block(b):
                s = _Blk()
                s.rows = slice(b * P, (b + 1) * P)
                s.coff = [0]
                for w in TILE_WS:
                    s.coff.append(s.coff[-1] + 8 * GCFG[w][1])
                s.xt = []
                return s

            def load(s, name):
                with nc.named_scope(f"load{name}"):
                    off = 0
                    for w in TILE_WS:
                        xt = dpool.tile([P, w], f16, tag=f"xt{w}", name="xt")
                        s.xt.append(xt)
                        nc.gpsimd.dma_start(out=xt,
                                            in_=x[s.rows, off:off + w])
                        off += w

            def alloc_warm(s):
                s.G = gpool.tile([P, 8000], f16, tag="G", name="G")
                s.VK = spool.tile([P, K], f16, tag="VK", name="VK")
                s.VKf = sm("VKf", K)
                s.rV, s.rV2 = sm("rV", K), sm("rV2", K)
                s.S, s.Q, s.rs, s.u, s.C = (sm("S"), sm("Q"), sm("rs"),
                                            sm("u"), sm("C"))
                s.nrsig = sm("nrsig")

            def fold_tile(s, t, name):
                """Fold tile t by pairwise halving into G, then max8 the
                subranges into the candidate buffer."""
                with nc.named_scope(f"fold{name}{t}"):
                    G = s.G
                    w = TILE_WS[t]
                    gw, nrg = GCFG[w]
                    h = w // 2
                    nc.vector.tensor_tensor(out=G[:, 0:h], in0=s.xt[t][:, 0:h],
                                            in1=s.xt[t][:, h:w], op=Alu.max)
                    while h > gw:
                        nh_ = h // 2
                        nc.vector.tensor_tensor(out=G[:, 0:nh_],
                                                in0=G[:, 0:nh_],
                                                in1=G[:, nh_:h], op=Alu.max)
                        h = nh_
                    W = gw // nrg
                    for i in range(nrg):
                        o = s.coff[t] + 8 * i
                        nc.vector.max(out=s.VK[:, o:o + 8],
                                      in_=G[:, W * i:W * (i + 1)])

            def warm_iters(s, width, iters):
                VKf, rV, rV2 = s.VKf, s.rV, s.rV2
                S, Q, rs, u, C = s.S, s.Q, s.rs, s.u, s.C
                for _ in range(iters):
                    nc.vector.scalar_tensor_tensor(
                        out=rV[:, :width], in0=VKf[:, :width], scalar=C,
                        in1=zb[:, :width], op0=Alu.subtract, op1=Alu.max,
                        accum_out=S)
                    nc.vector.scalar_tensor_tensor(
                        out=rV2[:, :width], in0=rV[:, :width], scalar=1.0,
                        in1=rV[:, :width], op0=Alu.mult, op1=Alu.mult,
                        accum_out=Q)
                    nc.vector.reciprocal(rs, S)
                    nc.vector.scalar_tensor_tensor(
                        out=u, in0=Q, scalar=4.0, in1=rs,
                        op0=Alu.subtract, op1=Alu.mult)
                    nc.vector.scalar_tensor_tensor(
                        out=C, in0=u, scalar=0.5, in1=C,
                        op0=Alu.mult, op1=Alu.add)

            def warm_pre(s, name):
                with nc.named_scope(f"warmpre{name}"):
                    nc.vector.tensor_copy(s.VKf[:, :K_PRE], s.VK[:, :K_PRE])
                    vsum = sm("vsum")
                    nc.vector.tensor_reduce(out=vsum, in_=s.VKf[:, :K_PRE],
                                            axis=AxX, op=Alu.add)
                    nc.vector.tensor_scalar_mul(s.C, vsum, 1.0 / K_PRE)
                    warm_iters(s, K_PRE, WARM_PRE)

            def warm_post(s, name):
                with nc.named_scope(f"warm{name}"):
                    nc.vector.tensor_copy(s.VKf, s.VK)
                    warm_iters(s, K, WARM_POST)
                    nc.vector.tensor_scalar_mul(s.nrsig, s.rs, -1.0)
                    s.cw = s.C

            def relu_and_f0(s, name):
                """DVE relu per unit; f0 accum per unit on ScalarE (Square)
                or DVE (stt square), unit assignment via F0_V_UNITS."""
                with nc.named_scope(f"iter{name}"):
                    nu = len(OUT_UNITS)
                    f0c = sm("f0c", nu)
                    s.f0c = f0c
                    for ui, (t, lo, w) in enumerate(OUT_UNITS):
                        sl = slice(lo, lo + w)
                        nc.vector.tensor_scalar(
                            out=s.xt[t][:, sl], in0=s.xt[t][:, sl],
                            scalar1=s.cw, scalar2=0.0,
                            op0=Alu.subtract, op1=Alu.max)
                        if ui in F0_V_UNITS:
                            continue
                        if w >= 6000:
                            tr = ypool.tile([P, 8000], f16, tag="ybtrash",
                                            name="ybtrash", bufs=1)[:, :w]
                        else:
                            tr = ppool.tile([P, w], f32, tag="ps", name="ps")
                        nc.scalar.activation(
                            out=tr, in_=s.xt[t][:, sl],
                            func=Act.Square, scale=0.5,
                            accum_out=f0c[:, ui:ui + 1])
                    for ui in F0_V_UNITS:
                        t, lo, w = OUT_UNITS[ui]
                        sl = slice(lo, lo + w)
                        gdst = s.G[:, 0:w]
                        nc.vector.scalar_tensor_tensor(
                            out=gdst, in0=s.xt[t][:, sl], scalar=0.25,
                            in1=s.xt[t][:, sl], op0=Alu.mult, op1=Alu.mult,
                            accum_out=f0c[:, ui:ui + 1])

            def newton(s, name):
                with nc.named_scope(f"newt{name}"):
                    f0 = sm("f0")
                    nc.vector.tensor_reduce(out=f0, in_=s.f0c, axis=AxX,
                                            op=Alu.add)
                    dc0, dc, nh = sm("dc0"), sm("dc"), sm("nh")
                    nc.scalar.activation(out=dc0, in_=f0, func=Act.Identity,
                                         scale=s.rs, bias=s.nrsig)
                    nc.scalar.activation(out=dc, in_=dc0, func=Act.Relu,
                                         scale=2.0)
                    nc.scalar.activation(out=nh, in_=dc, func=Act.Identity,
                                         scale=-0.5)
                    s.dc, s.nh = dc, nh

            def out_scalar(s, name, units):
                with nc.named_scope(f"out{name}"):
                    for (t, lo, w) in units:
                        sl = slice(lo, lo + w)
                        glo = sum(TILE_WS[:t]) + lo
                        yb = ypool.tile([P, 8000], f16, tag="yb", name="yb")
                        nc.scalar.activation(out=yb[:, :w],
                                             in_=s.xt[t][:, sl],
                                             func=Act.Square, scale=0.5,
                                             bias=s.nh)
                        nc.sync.dma_start(out=y[s.rows, glo:glo + w],
                                          in_=yb[:, :w])

            def out_dve(s, name, units):
                with nc.named_scope(f"out{name}"):
                    for (t, lo, w) in units:
                        sl = slice(lo, lo + w)
                        glo = sum(TILE_WS[:t]) + lo
                        nc.vector.tensor_scalar(
                            out=s.xt[t][:, sl], in0=s.xt[t][:, sl],
                            scalar1=s.dc, scalar2=0.5,
                            op0=Alu.subtract, op1=Alu.mult)
                        nc.vector.tensor_tensor(
                            out=s.xt[t][:, sl], in0=s.xt[t][:, sl],
                            in1=s.xt[t][:, sl], op=Alu.mult)
                        nc.sync.dma_start(out=y[s.rows, glo:glo + w],
                                          in_=s.xt[t][:, sl])

            A, B = new_block(0), new_block(1)
            load(A, "A")
            alloc_warm(A)
            fold_tile(A, 0, "A")       # fold head: tiles 0-1 + pre-warm
            fold_tile(A, 1, "A")
            load(B, "B")
            warm_pre(A, "A")
            fold_tile(A, 2, "A")       # fold tail: tiles 2-3
            fold_tile(A, 3, "A")
            warm_post(A, "A")
            relu_and_f0(A, "A")
            alloc_warm(B)
            fold_tile(B, 0, "B")       # B head
            fold_tile(B, 1, "B")
            newton(A, "A")             # f0 reduce (V) + newton smalls (S)
            out_scalar(A, "A", OUT_UNITS[:OUT_S_A])
            warm_pre(B, "B")
            fold_tile(B, 2, "B")       # B tail
            fold_tile(B, 3, "B")
            warm_post(B, "B")
            out_dve(A, "A", OUT_UNITS[OUT_S_A:])
            relu_and_f0(B, "B")
            newton(B, "B")
            out_dve(B, "B", sorted(OUT_UNITS[OUT_S_B:], key=lambda u: u[2]))
            out_scalar(B, "B", OUT_UNITS[:OUT_S_B])
    return nc


_COMPILED = {}


def _get_nc():
    if "nc" not in _COMPILED:
        nc = bacc.Bacc("TRN2", target_bir_lowering=False, debug=False,
                       num_devices=N_CORES)
        build_kernel(nc)
        nc.compile()
        _COMPILED["nc"] = nc
    return _COMPILED["nc"]


def kernel(X: np.ndarray) -> np.ndarray:
    assert X.shape == (ROWS_TOTAL, V) and X.dtype == np.float32, (X.shape, X.dtype)
    nc = _get_nc()
    in_maps = [
        {"x": np.ascontiguousarray(X[i * ROWS_PER_CORE:(i + 1) * ROWS_PER_CORE])}
        for i in range(N_CORES)
    ]
    res = run_bass_kernel_spmd(nc, in_maps, core_ids=list(range(N_CORES)))
    return np.concatenate(
        [r["y"].astype(np.float32) for r in res.results], axis=0)
